# revision 3
# baseline (speedup 1.0000x reference)
"""LoRA linear kernel for Trainium2 (8 NeuronCores, SPMD data-parallel).

Computes y = x @ (B @ A)^T for
    x: [4, 2048, 4096] f32, B: [4096, 16] f32, A: [16, 4096] f32.

Strategy: never materialize W = B @ A.  Factor as t = x @ A^T (rank 16)
then y = t @ B^T.  Tokens (4*2048 = 8192) are sharded across 8 cores
(1024 tokens each); A and B are replicated.  bf16 on the wire both ways
(rel err ~5e-3 vs the 2e-2 gate).

v3 schedule (trace-driven; v1 63.0us, v2 63.9us):
  - graded exec window = [first framework memset (~6.3us), last
    teardown instruction]; the ~8.7us teardown (sem resets from the
    NEFF wrapper) is fixed, so the lever is landing the LAST y write
    early.
  - x chunks (8x 1MB) prefetched on the SP (sync) HWDGE ring; one ring
    sustains ~420-450 GB/s (fabric ceiling ~435/NC).  Reads drain by
    ~28.5us.
  - consts (at, bt) ride the Scalar ring: they are small-descriptor
    transfers (~2us on the wire) and in v2 they sat at the head of the
    SP ring FIFO, delaying chunk0 ~3us, idling the PE and triggering a
    HAM down-clock (8/8 -> 4/8) right when mm1 started.
  - 4 groups of 256 tokens (was 2x512): mm1(g) completes right after
    its 2 chunks land, so mm2/evac/write production is spread across
    the whole read phase instead of compressed after 20us.  mm1 octet
    matmuls have free dim 256 (LDWEIGHTS ~98ns pipelines under the
    ~108ns rhs stream, so the PE cost increase is modest).
  - PSUM evacuation ([128,512] f32 PSUM -> bf16 SBUF, ~650-750ns) is
    split 4 DVE : 4 ACT per y chunk (GPSIMD has no PSUM port); the y
    dma_start issue (~590ns) is moved OFF the ACT queue: even chunks
    issue from the Sync queue (SP HWDGE ring), odd chunks from GpSimd
    (SWDGE) to test >1-ring aggregate write bandwidth.
  - HAM: PE clock starts 4/8; ~6 junk matmuls on a memset tile ramp it
    to 8/8 before chunk0 lands.
"""

import sys

import numpy as np

if "/opt/trn_rl_repo" not in sys.path:
    sys.path.insert(0, "/opt/trn_rl_repo")

# Problem shape (hardcoded per contract)
BATCH = 4
SEQ = 2048
D = 4096          # in_features == out_features
R = 16            # lora rank
NCORES = 8
NTOK = BATCH * SEQ            # 8192 tokens total
TOK = NTOK // NCORES          # 1024 tokens per core
P = 128                       # partitions
KO = D // P                   # 32 feature chunks
TB = 256                      # tokens per mm1 group (matmul free dim)
NG = TOK // TB                # 4 groups per core
NCHG = 2                      # x DMA chunks per group (1MB each)
KOC = KO // NCHG              # 16 ko-slices per chunk
NB = 512                      # matmul free dim for mm2 (psum bank limit)
NYC = TB // P                 # y chunks (128 tokens) per group

# Module-level knobs for test.py (harness never touches these)
TRACE = False
LAST_RESULTS = None
WRITE_RING = "split"          # "sync" | "split" (odd y chunks via SWDGE)

_nc_cache = None


def _build_program():
    from concourse import bacc, mybir, tile

    nc = bacc.Bacc(
        "TRN2", target_bir_lowering=False, debug=False, num_devices=NCORES
    )

    f32 = mybir.dt.float32
    bf16 = mybir.dt.bfloat16

    xt = nc.dram_tensor("xt", [NG, NCHG, P, KOC, TB], bf16, kind="ExternalInput")
    at = nc.dram_tensor("at", [P, KO, R], bf16, kind="ExternalInput")
    bt = nc.dram_tensor("bt", [R, D], bf16, kind="ExternalInput")
    y = nc.dram_tensor("y", [TOK, D], bf16, kind="ExternalOutput")

    with tile.TileContext(nc) as tc:
        with (
            tc.tile_pool(name="consts", bufs=1) as consts,
            tc.tile_pool(name="xin", bufs=NG * NCHG) as xin,
            tc.tile_pool(name="tbuf", bufs=2) as tbuf,
            tc.tile_pool(name="yout", bufs=8) as yout,
            tc.tile_pool(name="pt", bufs=2, space="PSUM") as pt_pool,
            tc.tile_pool(name="py", bufs=6, space="PSUM") as py_pool,
        ):
            # consts on the ACT (scalar) HWDGE ring so the SP ring head
            # is free for x chunk 0 immediately.
            at_s = consts.tile([P, KO, R], bf16)
            nc.scalar.dma_start(at_s[:], at[:])
            bt_s = consts.tile([R, D], bf16)
            nc.scalar.dma_start(bt_s[:], bt[:])

            # HAM pre-warm: junk matmuls gated only on the memset ramp
            # the PE clock to 8/8 during the DMA prologue.
            junk = consts.tile([P, NB], bf16)
            nc.gpsimd.memset(junk[:], 0.0)

            def pe_warm(n):
                for _ in range(n):
                    warm = py_pool.tile([P, NB], f32, tag="psum_y")
                    nc.tensor.matmul(
                        warm[:], junk[:, :P], junk[:],
                        start=True, stop=True, skip_group_check=True,
                    )

            pe_warm(6)
            tc.no_sync_barrier()

            # Prefetch every x chunk up front on the SP ring (8MB).
            xts = {}
            for g in range(NG):
                for c in range(NCHG):
                    t_ = xin.tile([P, KOC, TB], bf16, tag="xt")
                    nc.sync.dma_start(t_[:], xt[g, c])
                    xts[(g, c)] = t_

            def mm1_octet(g, c, psum_t):
                # one 1MB x chunk -> KOC accumulating matmuls
                xt_tile = xts[(g, c)]
                for j in range(KOC):
                    ko = c * KOC + j
                    nc.tensor.matmul(
                        psum_t[:, :TB],
                        at_s[:, ko, :],
                        xt_tile[:, j, :],
                        start=(ko == 0),
                        stop=(ko == KO - 1),
                        skip_group_check=True,
                    )

            def make_tT(psum_t):
                tT = tbuf.tile([R, TB], bf16)
                nc.vector.tensor_copy(tT[:], psum_t[:, :TB])
                return tT

            ychunk_idx = [0]

            def mm2_chunk(g, c, tT):
                y_row = yout.tile([P, D], bf16)
                for n in range(D // NB):
                    psum_y = py_pool.tile([P, NB], f32, tag="psum_y")
                    nc.tensor.matmul(
                        psum_y[:],
                        tT[:, c * P : (c + 1) * P],
                        bt_s[:, n * NB : (n + 1) * NB],
                        start=True,
                        stop=True,
                        skip_group_check=True,
                    )
                    # Single-bank PSUM evacuation, DVE 4 : ACT 4
                    if n % 2 == 0:
                        nc.vector.tensor_copy(y_row[:, n * NB : (n + 1) * NB], psum_y[:])
                    else:
                        nc.scalar.copy(y_row[:, n * NB : (n + 1) * NB], psum_y[:])
                row0 = g * TB + c * P
                k = ychunk_idx[0]
                ychunk_idx[0] += 1
                if WRITE_RING == "split" and k % 2 == 1:
                    nc.gpsimd.dma_start(y[row0 : row0 + P, :], y_row[:])
                else:
                    nc.sync.dma_start(y[row0 : row0 + P, :], y_row[:])

            # ---- schedule: sequential groups; production spread ----
            for g in range(NG):
                # psum_t is a full 2KB bank; matmuls touch [:, :TB]
                psum_t = pt_pool.tile([R, NB], f32, tag="psum_t")
                for c in range(NCHG):
                    mm1_octet(g, c, psum_t)
                tT = make_tT(psum_t)
                for c in range(NYC):
                    mm2_chunk(g, c, tT)

    nc.finalize()
    return nc


def kernel(x, lora_matrix_B, lora_matrix_A):
    global _nc_cache, LAST_RESULTS
    import ml_dtypes
    from concourse.bass_utils import run_bass_kernel_spmd

    if _nc_cache is None:
        _nc_cache = _build_program()
    nc = _nc_cache

    bf16 = ml_dtypes.bfloat16
    x_flat = np.asarray(x, dtype=np.float32).reshape(NTOK, D).astype(bf16)
    A = np.asarray(lora_matrix_A, dtype=np.float32).astype(bf16)
    B = np.asarray(lora_matrix_B, dtype=np.float32).astype(bf16)

    # at[p, ko, j] = A[j, ko*128 + p];  bt[j, o] = B[o, j]
    at_prep = np.ascontiguousarray(A.reshape(R, KO, P).transpose(2, 1, 0))
    bt_prep = np.ascontiguousarray(B.T)

    in_maps = []
    for core in range(NCORES):
        xc = x_flat[core * TOK : (core + 1) * TOK, :]
        # xt[g, c, p, j, t] = xc[g*TB + t, (c*KOC + j)*128 + p]
        xt_prep = np.ascontiguousarray(
            xc.reshape(NG, TB, NCHG, KOC, P).transpose(0, 2, 4, 3, 1)
        )
        in_maps.append({"xt": xt_prep, "at": at_prep, "bt": bt_prep})

    res = run_bass_kernel_spmd(
        nc, in_maps, core_ids=list(range(NCORES)), trace=TRACE
    )
    LAST_RESULTS = res

    y = np.concatenate([res.results[c]["y"] for c in range(NCORES)], axis=0)
    return y.reshape(BATCH, SEQ, D).astype(np.float32)


# revision 4
# speedup vs baseline: 1.0352x; 1.0352x over previous
"""LoRA linear kernel for Trainium2 (8 NeuronCores, SPMD data-parallel).

Computes y = x @ (B @ A)^T for
    x: [4, 2048, 4096] f32, B: [4096, 16] f32, A: [16, 4096] f32.

Strategy: never materialize W = B @ A.  Factor as t = x @ A^T (rank 16)
then y = t @ B^T.  Tokens (4*2048 = 8192) are sharded across 8 cores
(1024 tokens each); A and B are replicated.  bf16 on the wire both ways
(rel err ~5e-3 vs the 2e-2 gate).

v3 schedule (trace-driven; v1 63.0us, v2 63.9us):
  - graded exec window = [first framework memset (~6.3us), last
    teardown instruction]; the ~8.7us teardown (sem resets from the
    NEFF wrapper) is fixed, so the lever is landing the LAST y write
    early.
  - x chunks (8x 1MB) prefetched on the SP (sync) HWDGE ring; one ring
    sustains ~420-450 GB/s (fabric ceiling ~435/NC).  Reads drain by
    ~28.5us.
  - consts (at, bt) ride the Scalar ring: they are small-descriptor
    transfers (~2us on the wire) and in v2 they sat at the head of the
    SP ring FIFO, delaying chunk0 ~3us, idling the PE and triggering a
    HAM down-clock (8/8 -> 4/8) right when mm1 started.
  - 4 groups of 256 tokens (was 2x512): mm1(g) completes right after
    its 2 chunks land, so mm2/evac/write production is spread across
    the whole read phase instead of compressed after 20us.  mm1 octet
    matmuls have free dim 256 (LDWEIGHTS ~98ns pipelines under the
    ~108ns rhs stream, so the PE cost increase is modest).
  - PSUM evacuation ([128,512] f32 PSUM -> bf16 SBUF, ~650-750ns) is
    split 4 DVE : 4 ACT per y chunk (GPSIMD has no PSUM port); the y
    dma_start issue (~590ns) is moved OFF the ACT queue: even chunks
    issue from the Sync queue (SP HWDGE ring), odd chunks from GpSimd
    (SWDGE) to test >1-ring aggregate write bandwidth.
  - HAM: PE clock starts 4/8; ~6 junk matmuls on a memset tile ramp it
    to 8/8 before chunk0 lands.
"""

import sys

import numpy as np

if "/opt/trn_rl_repo" not in sys.path:
    sys.path.insert(0, "/opt/trn_rl_repo")

# Problem shape (hardcoded per contract)
BATCH = 4
SEQ = 2048
D = 4096          # in_features == out_features
R = 16            # lora rank
NCORES = 8
NTOK = BATCH * SEQ            # 8192 tokens total
TOK = NTOK // NCORES          # 1024 tokens per core
P = 128                       # partitions
KO = D // P                   # 32 feature chunks
TB = 256                      # tokens per mm1 group (matmul free dim)
NG = TOK // TB                # 4 groups per core
NCHG = 2                      # x DMA chunks per group (1MB each)
KOC = KO // NCHG              # 16 ko-slices per chunk
NB = 512                      # matmul free dim for mm2 (psum bank limit)
NYC = TB // P                 # y chunks (128 tokens) per group

# Module-level knobs for test.py (harness never touches these)
TRACE = False
LAST_RESULTS = None
WRITE_RING = "sync"           # "sync" | "split" (odd y chunks via SWDGE)

_nc_cache = None


def _build_program():
    from concourse import bacc, mybir, tile

    nc = bacc.Bacc(
        "TRN2", target_bir_lowering=False, debug=False, num_devices=NCORES
    )

    f32 = mybir.dt.float32
    bf16 = mybir.dt.bfloat16

    xt = nc.dram_tensor("xt", [NG, NCHG, P, KOC, TB], bf16, kind="ExternalInput")
    at = nc.dram_tensor("at", [P, KO, R], bf16, kind="ExternalInput")
    bt = nc.dram_tensor("bt", [R, D], bf16, kind="ExternalInput")
    y = nc.dram_tensor("y", [TOK, D], bf16, kind="ExternalOutput")

    with tile.TileContext(nc) as tc:
        with (
            tc.tile_pool(name="consts", bufs=1) as consts,
            tc.tile_pool(name="xin", bufs=NG * NCHG) as xin,
            tc.tile_pool(name="tbuf", bufs=2) as tbuf,
            tc.tile_pool(name="yout", bufs=8) as yout,
            tc.tile_pool(name="pt", bufs=1, space="PSUM") as pt_pool,
            tc.tile_pool(name="py", bufs=7, space="PSUM") as py_pool,
        ):
            # consts on the ACT (scalar) HWDGE ring so the SP ring head
            # is free for x chunk 0 immediately.
            at_s = consts.tile([P, KO, R], bf16)
            nc.scalar.dma_start(at_s[:], at[:])
            bt_s = consts.tile([R, D], bf16)
            nc.scalar.dma_start(bt_s[:], bt[:])

            # HAM pre-warm: junk matmuls gated only on the memset ramp
            # the PE clock to 8/8 during the DMA prologue.
            junk = consts.tile([P, NB], bf16)
            nc.gpsimd.memset(junk[:], 0.0)

            def pe_warm(n):
                for _ in range(n):
                    warm = py_pool.tile([P, NB], f32, tag="psum_y")
                    nc.tensor.matmul(
                        warm[:], junk[:, :P], junk[:],
                        start=True, stop=True, skip_group_check=True,
                    )

            pe_warm(7)
            tc.no_sync_barrier()

            # Prefetch every x chunk up front on the SP ring (8MB).
            xts = {}
            for g in range(NG):
                for c in range(NCHG):
                    t_ = xin.tile([P, KOC, TB], bf16, tag="xt")
                    nc.sync.dma_start(t_[:], xt[g, c])
                    xts[(g, c)] = t_

            def mm1_octet(g, c, psum_t):
                # one 1MB x chunk -> KOC accumulating matmuls
                xt_tile = xts[(g, c)]
                for j in range(KOC):
                    ko = c * KOC + j
                    nc.tensor.matmul(
                        psum_t[:, :TB],
                        at_s[:, ko, :],
                        xt_tile[:, j, :],
                        start=(ko == 0),
                        stop=(ko == KO - 1),
                        skip_group_check=True,
                    )

            def make_tT(psum_t):
                tT = tbuf.tile([R, TB], bf16)
                nc.vector.tensor_copy(tT[:], psum_t[:, :TB])
                return tT

            ychunk_idx = [0]

            def mm2_chunk(g, c, tT):
                y_row = yout.tile([P, D], bf16)
                for n in range(D // NB):
                    psum_y = py_pool.tile([P, NB], f32, tag="psum_y")
                    nc.tensor.matmul(
                        psum_y[:],
                        tT[:, c * P : (c + 1) * P],
                        bt_s[:, n * NB : (n + 1) * NB],
                        start=True,
                        stop=True,
                        skip_group_check=True,
                    )
                    # Single-bank PSUM evacuation, DVE 4 : ACT 4
                    if n % 2 == 0:
                        nc.vector.tensor_copy(y_row[:, n * NB : (n + 1) * NB], psum_y[:])
                    else:
                        nc.scalar.copy(y_row[:, n * NB : (n + 1) * NB], psum_y[:])
                row0 = g * TB + c * P
                k = ychunk_idx[0]
                ychunk_idx[0] += 1
                if WRITE_RING == "split" and k % 2 == 1:
                    nc.gpsimd.dma_start(y[row0 : row0 + P, :], y_row[:])
                else:
                    nc.sync.dma_start(y[row0 : row0 + P, :], y_row[:])

            # ---- schedule: sequential groups; production spread ----
            for g in range(NG):
                # psum_t is a full 2KB bank; matmuls touch [:, :TB]
                psum_t = pt_pool.tile([R, NB], f32, tag="psum_t")
                for c in range(NCHG):
                    mm1_octet(g, c, psum_t)
                tT = make_tT(psum_t)
                pe_warm(2)
                for c in range(NYC):
                    mm2_chunk(g, c, tT)

    nc.finalize()
    return nc


def kernel(x, lora_matrix_B, lora_matrix_A):
    global _nc_cache, LAST_RESULTS
    import ml_dtypes
    from concourse.bass_utils import run_bass_kernel_spmd

    if _nc_cache is None:
        _nc_cache = _build_program()
    nc = _nc_cache

    bf16 = ml_dtypes.bfloat16
    x_flat = np.asarray(x, dtype=np.float32).reshape(NTOK, D).astype(bf16)
    A = np.asarray(lora_matrix_A, dtype=np.float32).astype(bf16)
    B = np.asarray(lora_matrix_B, dtype=np.float32).astype(bf16)

    # at[p, ko, j] = A[j, ko*128 + p];  bt[j, o] = B[o, j]
    at_prep = np.ascontiguousarray(A.reshape(R, KO, P).transpose(2, 1, 0))
    bt_prep = np.ascontiguousarray(B.T)

    in_maps = []
    for core in range(NCORES):
        xc = x_flat[core * TOK : (core + 1) * TOK, :]
        # xt[g, c, p, j, t] = xc[g*TB + t, (c*KOC + j)*128 + p]
        xt_prep = np.ascontiguousarray(
            xc.reshape(NG, TB, NCHG, KOC, P).transpose(0, 2, 4, 3, 1)
        )
        in_maps.append({"xt": xt_prep, "at": at_prep, "bt": bt_prep})

    res = run_bass_kernel_spmd(
        nc, in_maps, core_ids=list(range(NCORES)), trace=TRACE
    )
    LAST_RESULTS = res

    y = np.concatenate([res.results[c]["y"] for c in range(NCORES)], axis=0)
    return y.reshape(BATCH, SEQ, D).astype(np.float32)
